# Initial kernel scaffold
#
"""FCGAT kernel for Trainium2 (8 NeuronCores, SPMD data-parallel over graphs).

The reference computes
    h   = x @ W_w.T + W_b                     [N,K,D]
    e   = leaky_relu(s_src[:,:,None] + s_dst[:,None,:] + b)
    a   = softmax(e, axis=2)                  [N,K,K]
    out = relu(einsum('nkj,nkd->nkd', a, h))
The einsum contracts the softmax over its own normalization axis, so
sum_j a[n,k,j] == 1 exactly and the whole attention block is an identity
scaling.  Hence out == relu(x @ W_w.T + W_b), which this kernel computes.

Device layout: each core gets 8 graphs (4096 rows).  Activations are staged
host-side as x^T [D, 4096] so the contraction dim lands on SBUF partitions
with no on-device transpose; the kernel emits out^T [D, 4096] which the host
transposes back during unsharding.
"""

import numpy as np

N, K, D = 64, 512, 256
N_CORES = 8
G_PER_CORE = N // N_CORES          # 8 graphs per core
TOK = G_PER_CORE * K               # 4096 rows per core
P = 128                            # SBUF partitions
BLK = 512                          # moving-operand free dim per matmul (fp32 max)

_cached = {}


def _build_nc():
    import concourse.bass as bass
    import concourse.mybir as mybir
    import concourse.tile as tile

    f32 = mybir.dt.float32
    nc = bass.Bass("TRN2", target_bir_lowering=False, debug=False)

    xT = nc.dram_tensor("xT", [D, TOK], f32, kind="ExternalInput").ap()
    wT = nc.dram_tensor("wT", [D, D], f32, kind="ExternalInput").ap()
    bias = nc.dram_tensor("bias", [D, 1], f32, kind="ExternalInput").ap()
    outT = nc.dram_tensor("outT", [D, TOK], f32, kind="ExternalOutput").ap()

    nblk = TOK // BLK
    with tile.TileContext(nc) as tc:
        with (
            tc.tile_pool(name="wp", bufs=1) as wp,
            tc.tile_pool(name="xp", bufs=4) as xp,
            tc.tile_pool(name="op", bufs=4) as op,
            tc.tile_pool(name="pp", bufs=4, space="PSUM") as pp,
        ):
            # wT[d,e] staged as [128, 2*D]: cols 0:D hold d-chunk 0, D:2D d-chunk 1
            w_sb = wp.tile([P, 2 * D], f32)
            nc.sync.dma_start(w_sb[:, 0:D], wT[0:P, :])
            nc.sync.dma_start(w_sb[:, D : 2 * D], wT[P : 2 * P, :])
            b_sb = wp.tile([P, 2], f32)
            nc.sync.dma_start(b_sb[:, 0:1], bias[0:P, :])
            nc.sync.dma_start(b_sb[:, 1:2], bias[P : 2 * P, :])

            for blk in range(nblk):
                cs = slice(blk * BLK, (blk + 1) * BLK)
                x0 = xp.tile([P, BLK], f32, tag="x")
                nc.sync.dma_start(x0[:], xT[0:P, cs])
                x1 = xp.tile([P, BLK], f32, tag="x")
                nc.sync.dma_start(x1[:], xT[P : 2 * P, cs])
                for ec in range(2):
                    ps = pp.tile([P, BLK], f32, tag="ps")
                    nc.tensor.matmul(
                        ps[:], w_sb[:, ec * P : (ec + 1) * P], x0[:],
                        start=True, stop=False,
                    )
                    nc.tensor.matmul(
                        ps[:], w_sb[:, D + ec * P : D + (ec + 1) * P], x1[:],
                        start=False, stop=True,
                    )
                    o = op.tile([P, BLK], f32, tag="o")
                    nc.scalar.activation(
                        o[:], ps[:], mybir.ActivationFunctionType.Relu,
                        bias=b_sb[:, ec : ec + 1],
                    )
                    nc.sync.dma_start(outT[ec * P : (ec + 1) * P, cs], o[:])
    return nc


def kernel(x, W_w, W_b, att_w, att_b):
    from concourse.bass_utils import run_bass_kernel_spmd

    if "nc" not in _cached:
        _cached["nc"] = _build_nc()
    nc = _cached["nc"]

    x = np.ascontiguousarray(x, dtype=np.float32)
    wT = np.ascontiguousarray(np.asarray(W_w, dtype=np.float32).T)
    b = np.ascontiguousarray(np.asarray(W_b, dtype=np.float32).reshape(D, 1))

    in_maps = []
    for c in range(N_CORES):
        shard = x[c * G_PER_CORE : (c + 1) * G_PER_CORE].reshape(TOK, D)
        in_maps.append(
            {"xT": np.ascontiguousarray(shard.T), "wT": wT, "bias": b}
        )

    res = run_bass_kernel_spmd(nc, in_maps, core_ids=list(range(N_CORES)))

    out = np.empty((N, K, D), dtype=np.float32)
    for c in range(N_CORES):
        oT = res.results[c]["outT"]  # [D, TOK]
        out[c * G_PER_CORE : (c + 1) * G_PER_CORE] = oT.T.reshape(G_PER_CORE, K, D)
    return out


# revision 13
# speedup vs baseline: 1.2194x; 1.2194x over previous
"""FCGAT kernel for Trainium2 (8 NeuronCores, SPMD data-parallel over graphs).

The reference computes
    h   = x @ W_w.T + W_b                     [N,K,D]
    e   = leaky_relu(s_src[:,:,None] + s_dst[:,None,:] + b)
    a   = softmax(e, axis=2)                  [N,K,K]
    out = relu(einsum('nkj,nkd->nkd', a, h))
The einsum contracts the softmax over its own normalization axis, so
sum_j a[n,k,j] == 1 exactly and the whole attention block is an identity
scaling.  Hence out == relu(x @ W_w.T + W_b), which this kernel computes.

Device layout: each core gets 8 graphs (4096 rows).  Activations are staged
host-side as x^T [D, 4096] so the contraction dim lands on SBUF partitions
with no on-device transpose; the kernel emits out^T [D, 4096] which the host
transposes back during unsharding.  W^T and the bias are packed into one
[128, 2*D+2] params tensor so consumers depend on a single DMA.
"""

import numpy as np

N, K, D = 64, 512, 256
N_CORES = 8
G_PER_CORE = N // N_CORES          # 8 graphs per core
TOK = G_PER_CORE * K               # 4096 rows per core
P = 128                            # SBUF partitions
BLK = 512                          # moving-operand free dim per matmul (fp32 max)

_cached = {}


def _build_nc(mm_dtype="f32", repeats=1, loop_iters=1, xbufs=4, obufs=4, psbufs=4):
    import concourse.mybir as mybir
    import concourse.tile as tile
    from concourse import bacc

    f32 = mybir.dt.float32
    mmdt = {"f32": f32, "f32r": mybir.dt.float32r}[mm_dtype]
    nc = bacc.Bacc("TRN2", target_bir_lowering=False, debug=False)

    xT = nc.dram_tensor("xT", [2 * P, TOK], mmdt, kind="ExternalInput").ap()
    params = nc.dram_tensor("params", [P, 2 * D + 2], mmdt, kind="ExternalInput").ap()
    outT = nc.dram_tensor("outT", [2 * P, TOK], f32, kind="ExternalOutput").ap()

    xT_r = xT.rearrange("(c p) t -> p c t", p=P)  # d = c*128 + p

    nblk = TOK // BLK
    with tile.TileContext(nc) as tc:
        with (
            tc.tile_pool(name="wp", bufs=1) as wp,
            tc.tile_pool(name="xp", bufs=xbufs) as xp,
            tc.tile_pool(name="op", bufs=obufs) as op,
            tc.tile_pool(name="pp", bufs=psbufs, space="PSUM") as pp,
        ):
            # cols [0:256) = W^T rows d=0..127, [256:512) = d=128..255,
            # [512:514) = bias chunks (col c holds bias[c*128 + p])
            w_sb = wp.tile([P, 2 * D + 2], mmdt)
            nc.sync.dma_start(w_sb[:], params[:])
            w_mm = w_sb[:]
            w_bias = w_sb[:].bitcast(f32) if mm_dtype != "f32" else w_sb[:]

            import contextlib

            loop_cm = (
                tc.For_i(0, loop_iters, 1) if loop_iters > 1
                else contextlib.nullcontext()
            )
            with loop_cm:
                _emit_body(nc, tc, mm_dtype, repeats, xp, op, pp, w_mm, w_bias,
                           xT_r, outT, f32, mmdt)
    nc.compile()
    return nc


def _emit_body(nc, tc, mm_dtype, repeats, xp, op, pp, w_mm, w_bias, xT_r, outT,
               f32, mmdt):
    import concourse.mybir as mybir

    nblk = TOK // BLK
    for rep in range(repeats):
        for blk in range(nblk):
            cs = slice(blk * BLK, (blk + 1) * BLK)
            x_sb = xp.tile([P, 2 * BLK], mmdt, tag="x")
            nc.sync.dma_start(
                x_sb[:].rearrange("p (c t) -> p c t", c=2), xT_r[:, :, cs]
            )
            for ec in range(2):
                ps = pp.tile([P, BLK], f32, tag="ps")
                nc.tensor.matmul(
                    ps[:], w_mm[:, ec * P : (ec + 1) * P], x_sb[:, 0:BLK],
                    start=True, stop=False,
                )
                nc.tensor.matmul(
                    ps[:], w_mm[:, D + ec * P : D + (ec + 1) * P],
                    x_sb[:, BLK : 2 * BLK],
                    start=False, stop=True,
                )
                o = op.tile([P, BLK], f32, tag="o")
                nc.scalar.activation(
                    o[:], ps[:], mybir.ActivationFunctionType.Relu,
                    bias=w_bias[:, 2 * D + ec : 2 * D + ec + 1],
                )
                nc.sync.dma_start(outT[ec * P : (ec + 1) * P, cs], o[:])


def _prep_params(W_w, W_b):
    wT = np.asarray(W_w, dtype=np.float32).T  # wT[d, e] = W_w[e, d]
    bias_cols = np.asarray(W_b, dtype=np.float32).reshape(2, P).T  # [128, 2]
    return np.ascontiguousarray(
        np.concatenate([wT[0:P, :], wT[P : 2 * P, :], bias_cols], axis=1)
    )


def kernel(x, W_w, W_b, att_w, att_b):
    from concourse.bass_utils import run_bass_kernel_spmd

    if "nc" not in _cached:
        _cached["nc"] = _build_nc()
    nc = _cached["nc"]

    x = np.ascontiguousarray(x, dtype=np.float32)
    params = _prep_params(W_w, W_b)

    in_maps = []
    for c in range(N_CORES):
        shard = x[c * G_PER_CORE : (c + 1) * G_PER_CORE].reshape(TOK, D)
        in_maps.append({"xT": np.ascontiguousarray(shard.T), "params": params})

    res = run_bass_kernel_spmd(nc, in_maps, core_ids=list(range(N_CORES)))

    out = np.empty((N, K, D), dtype=np.float32)
    for c in range(N_CORES):
        oT = res.results[c]["outT"]  # [D, TOK]
        out[c * G_PER_CORE : (c + 1) * G_PER_CORE] = oT.T.reshape(G_PER_CORE, K, D)
    return out
